# revision 1
# baseline (speedup 1.0000x reference)
"""MDTA Bass kernel v3 — fp8 DoubleRow qk-path + bf16 banded v-path.

Per core = one batch image (data-parallel over B=8).

  qk path (fp8 errors wash out through normalization + gram averaging):
    1x1 conv via fp8 DoubleRow GEMM -> qp fp8 -> depthwise 3x3 via fp8 DR
    diagonal matmuls (2 taps/matmul, row-pair APs) -> bf16 -> PE transposes
    -> per-head gram [96,96] PSUM accumulation -> softmax on qk block.
  v path (bf16 end-to-end; errors here go straight to the output):
    transposed 1x1 conv (pvT [w, h, vc]) -> depthwise 3x3 as per-channel
    tridiagonal band matmuls (bands streamed from HBM under the slab loop)
    -> transpose back to channel-major -> out = (Wproj@blockdiag(A)) @ v
    in bf16 -> f16 output (host casts f32).
"""
import sys

for _p in ("/opt/trn_rl_repo", "/root/.axon_site/_ro/trn_rl_repo"):
    if _p not in sys.path:
        sys.path.insert(0, _p)

import numpy as np
import ml_dtypes

import bass_rust as _br
import concourse.bass as bass
from concourse import bacc, mybir
import concourse.tile as tile
from concourse.bass_utils import run_bass_kernel_spmd

F8 = mybir.dt.float8e4
F16 = mybir.dt.bfloat16
FH = mybir.dt.float16
F32 = mybir.dt.float32
DR = mybir.MatmulPerfMode.DoubleRow

B, C, HH, WW = 8, 192, 128, 128
N = HH * WW
HEADS, HD = 4, 48
C3 = 3 * C
RSLAB = 16
NSLAB = HH // RSLAB            # 8
NT = N // 512                  # 32
QROWS = RSLAB + 3              # qp rows: halo + junk guard row
NBCH = 24                      # band chunks (8 v-channels each)

DWP = [((0, 0), (1, 0)), ((0, 1), (1, 1)), ((0, 2), (1, 2)),
       ((2, 0), None), ((2, 1), None), ((2, 2), None)]

_CACHE = {}


def _row_groups(jlo, jhi):
    out = []
    j = jlo
    while j < jhi:
        out.append((j, min(j + 4, jhi)))
        j = min(j + 4, jhi)
    return out


def build_program(inv_temp: float, reps: int = 1):
    nc = bacc.Bacc("TRN2", target_bir_lowering=False, debug=False, num_devices=8)

    x8_d = nc.dram_tensor("x8", [96, 2, HH, WW], F8, kind="ExternalInput").ap()
    x16a_d = nc.dram_tensor("x16a", [128, N], F16, kind="ExternalInput").ap()
    x16b_d = nc.dram_tensor("x16b", [64, N], F16, kind="ExternalInput").ap()
    w8_d = nc.dram_tensor("w8", [96, 2, 384], F8, kind="ExternalInput").ap()
    wvta_d = nc.dram_tensor("wvta", [128, 192], F16, kind="ExternalInput").ap()
    wvtb_d = nc.dram_tensor("wvtb", [64, 192], F16, kind="ExternalInput").ap()
    lw_d = nc.dram_tensor("lwdw", [128, 3, 6, 2, 128], F8, kind="ExternalInput").ap()
    bands_d = nc.dram_tensor("bands", [128, 192, 3, 128], F16, kind="ExternalInput").ap()
    wjh_d = nc.dram_tensor("wjh", [48, 4, C], F16, kind="ExternalInput").ap()
    id16_d = nc.dram_tensor("id16", [128, 128], F16, kind="ExternalInput").ap()
    id96_d = nc.dram_tensor("id96", [96, 96], F32, kind="ExternalInput").ap()
    out_d = nc.dram_tensor("out", [C, N], FH, kind="ExternalOutput").ap()

    from contextlib import ExitStack  # noqa
    with tile.TileContext(nc) as tc:
        ev_state = [0]

        def evict(dst, src):
            e = ev_state[0]
            ev_state[0] = (e + 1) % 2
            if e == 0:
                nc.scalar.copy(dst, src)
            else:
                nc.vector.tensor_copy(dst, src)

        with tc.tile_pool(name="res", bufs=1) as res:
            x8 = res.tile([96, 2, HH, WW], F8, tag="x8")
            w8 = res.tile([96, 2, 384], F8, tag="w8")
            wvta = res.tile([128, 192], F16, tag="wvta")
            wvtb = res.tile([64, 192], F16, tag="wvtb")
            lw = res.tile([128, 3, 6, 2, 128], F8, tag="lw")
            wjh = res.tile([48, 4, C], F16, tag="wjh")
            id16 = res.tile([128, 128], F16, tag="id16")
            id96 = res.tile([96, 96], F32, tag="id96")
            vT = res.tile([128, 192, HH], F16, tag="vT")
            for t, d in ((wvta, wvta_d), (wvtb, wvtb_d), (id16, id16_d),
                         (id96, id96_d), (wjh, wjh_d)):
                nc.sync.dma_start(t[:], d[:])
            for rep in range(reps):
                p0 = ExitStack()
                big = p0.enter_context(tc.tile_pool(name="big", bufs=1))
                qpp = p0.enter_context(tc.tile_pool(name="qpp", bufs=2))
                qkdp = p0.enter_context(tc.tile_pool(name="qkdp", bufs=2))
                qktp = p0.enter_context(tc.tile_pool(name="qktp", bufs=3))
                bandp = p0.enter_context(tc.tile_pool(name="bandp", bufs=2))

                pvT = big.tile([128, 130, 192], F16, tag="pvT")   # [w, h+pad, vc]

                # ---------------- pre-pass: transposed 1x1 conv for v ----------------
                with tc.tile_pool(name="x16p", bufs=2) as x16p, \
                     tc.tile_pool(name="pvps", bufs=4, space="PSUM") as pvps:
                    nc.gpsimd.memset(pvT[:, 0, :], 0.0)
                    nc.gpsimd.memset(pvT[:, 129, :], 0.0)
                    for s in range(NSLAB):
                        c0 = s * RSLAB * 128
                        xa = x16p.tile([128, RSLAB * 128], F16, tag="xa", name=f"xa{s}_{rep}")
                        xb = x16p.tile([64, RSLAB * 128], F16, tag="xb", name=f"xb{s}_{rep}")
                        nc.sync.dma_start(xa[:], x16a_d[:, c0:c0 + RSLAB * 128])
                        nc.sync.dma_start(xb[:], x16b_d[:, c0:c0 + RSLAB * 128])
                        if rep == 0 and s == 1:
                            # bulk main-loop inputs; overlaps pre-pass compute
                            nc.sync.dma_start(x8[:], x8_d[:])
                            nc.sync.dma_start(w8[:], w8_d[:])
                            nc.sync.dma_start(lw[:], lw_d[:])
                        for i in range(RSLAB):
                            h = s * RSLAB + i
                            pc = pvps.tile([128, 192], F32, tag="pv", name=f"pv{h}_{rep}")
                            nc.tensor.matmul(pc[:], xa[:, 128 * i:128 * i + 128],
                                             wvta[:], start=True, stop=False)
                            nc.tensor.matmul(pc[:], xb[:, 128 * i:128 * i + 128],
                                             wvtb[:], start=False, stop=True)
                            evict(pvT[:, h + 1, :], pc[:])

                # gram accumulator (4 heads x [96,96] f32, one bank)
                p1 = ExitStack()
                pwps = p1.enter_context(tc.tile_pool(name="pwps", bufs=2, space="PSUM"))
                dwps = p1.enter_context(tc.tile_pool(name="dwps", bufs=3, space="PSUM"))
                tps = p1.enter_context(tc.tile_pool(name="tps", bufs=2, space="PSUM"))
                gps = p1.enter_context(tc.tile_pool(name="gps", bufs=1, space="PSUM"))
                G = gps.tile([96, 4, 96], F32, tag="G", name=f"G_{rep}")

                def emit_transgram_h(qkds, s_of, h):
                    pt = tps.tile([128, 384], F16, tag="pt", name=f"pt_{s_of}_{h}_{rep}")
                    for mi in range(3):
                        nc.tensor.transpose(pt[:, 128 * mi:128 * mi + 128],
                                            qkds[mi][:, h, :], id16[:])
                    qkt = qktp.tile([128, 384], F16, tag="qkt", name=f"qkt_{s_of}_{h}_{rep}")
                    nc.vector.tensor_copy(qkt[:], pt[:])
                    for p in range(HEADS):
                        first = s_of == 0 and h == 0 and p == 0
                        last = (s_of == NSLAB - 1 and h == RSLAB - 1
                                and p == HEADS - 1)
                        nc.tensor.matmul(G[:, p, :], qkt[:, 96 * p:96 * p + 96],
                                         qkt[:, 96 * p:96 * p + 96],
                                         start=first, stop=last)

                def emit_transgram(qkds, s_of):
                    for h in range(RSLAB):
                        emit_transgram_h(qkds, s_of, h)

                def emit_band_chunk(k):
                    """8 v-channels: dma bands, 24 band matmuls, 2 evicts to vT."""
                    bt = bandp.tile([128, 8, 3, 128], F16, tag="bt", name=f"bt{k}_{rep}")
                    nc.sync.dma_start(bt[:], bands_d[:, 8 * k:8 * k + 8, :, :])
                    for half in range(2):
                        ps0 = dwps.tile([128, 512], F32, tag="dw", name=f"bps{k}_{half}_{rep}")
                        ps = ps0.rearrange("p (c w) -> p c w", c=4)
                        for ci in range(4):
                            vc = 8 * k + 4 * half + ci
                            for dy in range(3):
                                nc.tensor.matmul(
                                    ps[:, ci, :], bt[:, 4 * half + ci, dy, :],
                                    pvT[:, dy:dy + 128, vc],
                                    start=(ci == 0 and dy == 0),
                                    stop=(ci == 3 and dy == 2))
                        if (k + half) % 2 == 0:
                            nc.vector.tensor_copy(
                                vT[:, 8 * k + 4 * half:8 * k + 4 * half + 4, :], ps0[:])
                        else:
                            nc.scalar.copy(
                                vT[:, 8 * k + 4 * half:8 * k + 4 * half + 4, :], ps0[:])

                pend = None
                for s in range(NSLAB):
                    r0 = RSLAB * s
                    jlo = 1 if s == 0 else 0
                    jhi = RSLAB + 1 if s == NSLAB - 1 else RSLAB + 2

                    qp = [qpp.tile([128, QROWS, 130], F8, tag=f"qp{m}",
                                   name=f"qp{m}_{s}_{rep}") for m in range(3)]
                    for m in range(3):
                        nc.gpsimd.memset(qp[m][:, :, 0:1], 0.0)
                        nc.gpsimd.memset(qp[m][:, :, 129:130], 0.0)
                        nc.gpsimd.memset(qp[m][:, jhi:QROWS, :], 0.0)
                        if s == 0:
                            nc.gpsimd.memset(qp[m][:, 0:1, :], 0.0)

                    # pointwise qk (fp8 DR)
                    for (ja, jb) in _row_groups(jlo, jhi):
                        nr = jb - ja
                        for m in range(3):
                            ps = pwps.tile([128, nr * 128], F32, tag="pw",
                                           name=f"pw_{s}_{ja}_{m}_{rep}")
                            nc.tensor.matmul(
                                ps[:], w8[:, :, 128 * m:128 * m + 128],
                                x8[:, :, r0 - 1 + ja:r0 - 1 + jb, :],
                                start=True, stop=True, perf_mode=DR)
                            if (m + ja) % 2 == 0:
                                nc.vector.tensor_copy(qp[m][:, ja:jb, 1:129], ps[:])
                            else:
                                nc.scalar.copy(qp[m][:, ja:jb, 1:129], ps[:])

                    # woven: dw groups + band chunks + trans/gram(s-1)
                    qkd = [qkdp.tile([128, RSLAB, 128], F16, tag=f"qkd{i}",
                                     name=f"qkd{i}_{s}_{rep}") for i in range(3)]
                    qp_pitch = QROWS * 130

                    def dw_group(i):
                        m, g = i // 4, i % 4
                        dps = dwps.tile([128, 512], F32, tag="dw",
                                        name=f"dw_{s}_{m}_{g}_{rep}")
                        for t, (tapa, tapb) in enumerate(DWP):
                            dy, dx = tapa
                            rhs = qp[m][:].copy()
                            rhs.ap = _br.VecI64Pair(
                                [[qp_pitch, 128], [130, 2], [130, 4], [1, 128]])
                            rhs.offset = qp[m][:].offset + (4 * g + dy) * 130 + dx
                            nc.tensor.matmul(dps[:], lw[:, m, t, :, :], rhs,
                                             start=(t == 0), stop=(t == 5),
                                             perf_mode=DR)
                        if (m + g) % 2 == 0:
                            nc.scalar.copy(qkd[m][:, 4 * g:4 * g + 4, :], dps[:])
                        else:
                            nc.vector.tensor_copy(qkd[m][:, 4 * g:4 * g + 4, :], dps[:])

                    bq = list(range(3 * s, 3 * s + 3))
                    for i in range(RSLAB):
                        if i < 12:
                            dw_group(i)
                        if i % 5 == 0 and bq:
                            emit_band_chunk(bq.pop(0))
                        if pend is not None:
                            emit_transgram_h(pend, s - 1, i)
                    pend = qkd

                emit_transgram(pend, NSLAB - 1)
                pend = None
                p0.close()

                # ---------------- softmax per head + WpA ----------------
                with tc.tile_pool(name="sm", bufs=1) as sm:
                    gs = sm.tile([96, 4, 96], F32, tag="gs")
                    nc.vector.tensor_copy(gs[:], G[:])
                    p1.close()

                    aw = sm.tile([48, 4, 48], F16, tag="aw")
                    with tc.tile_pool(name="smps", bufs=2, space="PSUM") as smps:
                        for p in range(HEADS):
                            gsp = gs[:, p, :]
                            dg = sm.tile([96, 96], F32, tag="dg", name=f"dg{p}_{rep}")
                            sd = sm.tile([96, 1], F32, tag="sd", name=f"sd{p}_{rep}")
                            nc.vector.tensor_mul(dg[:], gsp, id96[:])
                            nc.vector.reduce_sum(sd[:], dg[:], axis=mybir.AxisListType.X)
                            rr = sm.tile([96, 1], F32, tag="rr", name=f"rr{p}_{rep}")
                            nc.scalar.activation(rr[:], sd[:],
                                                 mybir.ActivationFunctionType.Sqrt,
                                                 scale=float(inv_temp))
                            nc.vector.reciprocal(rr[:], rr[:])
                            rrt = sm.tile([1, 96], F32, tag="rrt", name=f"rrt{p}_{rep}")
                            tp0 = smps.tile([1, 96], F32, tag="tp0", name=f"tp0{p}_{rep}")
                            nc.tensor.transpose(tp0[:], rr[:], id96[:])
                            nc.vector.tensor_copy(rrt[:], tp0[:])
                            ops = smps.tile([96, 96], F32, tag="ops", name=f"ops{p}_{rep}")
                            nc.tensor.matmul(ops[:], rrt[0:1, :], rrt[0:1, :],
                                             start=True, stop=True)
                            lg = sm.tile([48, 48], F32, tag="lg", name=f"lg{p}_{rep}")
                            nc.vector.tensor_mul(lg[:], gsp[0:48, 48:96],
                                                 ops[0:48, 48:96])
                            nmax = sm.tile([48, 1], F32, tag="nm", name=f"nm{p}_{rep}")
                            nc.vector.reduce_max(nmax[:], lg[:], axis=mybir.AxisListType.X)
                            nc.vector.tensor_scalar_mul(nmax[:], nmax[:], -1.0)
                            ex = sm.tile([48, 48], F32, tag="ex", name=f"ex{p}_{rep}")
                            nc.scalar.activation(ex[:], lg[:],
                                                 mybir.ActivationFunctionType.Exp,
                                                 bias=nmax[:])
                            rs = sm.tile([48, 1], F32, tag="rs", name=f"rs{p}_{rep}")
                            nc.vector.reduce_sum(rs[:], ex[:], axis=mybir.AxisListType.X)
                            nc.vector.reciprocal(rs[:], rs[:])
                            nc.vector.tensor_scalar_mul(aw[:, p, :], ex[:], rs[:])

                    # WpA = Wproj @ blockdiag(A), then (WpA)^T in bf16
                    was1 = sm.tile([128, 192], F16, tag="was1")
                    was2 = sm.tile([64, 192], F16, tag="was2")
                    watA = sm.tile([128, 192], F16, tag="watA")   # (WpA)^T rows 0:128
                    watB = sm.tile([64, 192], F16, tag="watB")    # rows 128:192
                    with tc.tile_pool(name="wpps", bufs=1, space="PSUM") as wpps:
                        wa1 = wpps.tile([128, 192], F32, tag="wa1", name=f"wa1_{rep}")
                        wa2 = wpps.tile([64, 192], F32, tag="wa2", name=f"wa2_{rep}")
                        for p in range(HEADS):
                            fi, la = p == 0, p == HEADS - 1
                            nc.tensor.matmul(wa1[:, 48 * p:48 * p + 48],
                                             wjh[:, p, 0:128], aw[:, p, :],
                                             start=fi, stop=la)
                            nc.tensor.matmul(wa2[:, 48 * p:48 * p + 48],
                                             wjh[:, p, 128:192], aw[:, p, :],
                                             start=fi, stop=la)
                        nc.vector.tensor_copy(was1[:], wa1[:])
                        nc.scalar.copy(was2[:], wa2[:])
                        tt1 = wpps.tile([128, 128], F16, tag="tt1", name=f"tt1_{rep}")
                        tt2 = wpps.tile([128, 64], F16, tag="tt2", name=f"tt2_{rep}")
                        tt3 = wpps.tile([64, 128], F16, tag="tt3", name=f"tt3_{rep}")
                        tt4 = wpps.tile([64, 64], F16, tag="tt4", name=f"tt4_{rep}")
                        nc.tensor.transpose(tt1[:], was1[:, 0:128], id16[:])
                        nc.tensor.transpose(tt2[:], was2[:, 0:128], id16[0:64, 0:64])
                        nc.tensor.transpose(tt3[:], was1[:, 128:192], id16[:])
                        nc.tensor.transpose(tt4[:], was2[:, 128:192], id16[0:64, 0:64])
                        nc.vector.tensor_copy(watA[:, 0:128], tt1[:])
                        nc.scalar.copy(watA[:, 128:192], tt2[:])
                        nc.vector.tensor_copy(watB[:, 0:128], tt3[:])
                        nc.scalar.copy(watB[:, 128:192], tt4[:])

                    # ---- transpose v back + out GEMM, interleaved ----
                    vcmA = sm.tile([128, N], F16, tag="vcmA")
                    vcmB = sm.tile([64, N], F16, tag="vcmB")
                    with tc.tile_pool(name="t2ps", bufs=2, space="PSUM") as t2ps, \
                         tc.tile_pool(name="avps", bufs=2, space="PSUM") as avps, \
                         tc.tile_pool(name="osb", bufs=4) as osbp:
                        def t2_chunk(nt):
                            for h in range(4 * nt, 4 * nt + 4):
                                pa = t2ps.tile([128, 128], F16, tag="pa", name=f"pa{h}_{rep}")
                                nc.tensor.transpose(pa[:], vT[:, 0:128, h], id16[:])
                                pb = t2ps.tile([64, 128], F16, tag="pb", name=f"pb{h}_{rep}")
                                nc.tensor.transpose(pb[:], vT[:, 128:192, h], id16[:])
                                evict(vcmA[:, 128 * h:128 * h + 128], pa[:])
                                evict(vcmB[:, 128 * h:128 * h + 128], pb[:])

                        t2_chunk(0)
                        for nt in range(NT):
                            if nt + 1 < NT:
                                t2_chunk(nt + 1)
                            col = 512 * nt
                            o1 = avps.tile([128, 512], F32, tag="o1")
                            nc.tensor.matmul(o1[:], watA[:, 0:128],
                                             vcmA[:, col:col + 512],
                                             start=True, stop=False)
                            nc.tensor.matmul(o1[:], watB[:, 0:128],
                                             vcmB[:, col:col + 512],
                                             start=False, stop=True)
                            o2 = avps.tile([64, 512], F32, tag="o2")
                            nc.tensor.matmul(o2[:], watA[:, 128:192],
                                             vcmA[:, col:col + 512],
                                             start=True, stop=False)
                            nc.tensor.matmul(o2[:], watB[:, 128:192],
                                             vcmB[:, col:col + 512],
                                             start=False, stop=True)
                            s1 = osbp.tile([128, 512], FH, tag="s1")
                            evict(s1[:], o1[:])
                            s2 = osbp.tile([64, 512], FH, tag="s2")
                            evict(s2[:], o2[:])
                            nc.sync.dma_start(out_d[0:128, col:col + 512], s1[:])
                            nc.sync.dma_start(out_d[128:192, col:col + 512], s2[:])

    nc.compile()
    return nc


def _perm_qk():
    p = []
    for h in range(HEADS):
        p.extend(range(h * HD, (h + 1) * HD))
        p.extend(range(C + h * HD, C + (h + 1) * HD))
    return np.asarray(p)


def _host_inputs(x, w_pw, w_dw, w_proj):
    f8 = ml_dtypes.float8_e4m3
    f16 = ml_dtypes.bfloat16
    perm = _perm_qk()
    wqk = w_pw[perm]                          # [384, 192]
    w8 = np.empty((96, 2, 384), np.float32)
    w8[:, 0, :] = wqk[:, 0:96].T
    w8[:, 1, :] = wqk[:, 96:192].T

    w9 = w_dw.reshape(C3, 9)
    w9qk = w9[perm]
    lw = np.zeros((128, 3, 6, 2, 128), np.float32)
    for m in range(3):
        for t, (tapa, tapb) in enumerate(DWP):
            ta = tapa[0] * 3 + tapa[1]
            lw[np.arange(128), m, t, 0, np.arange(128)] = w9qk[128 * m:128 * m + 128, ta]
            if tapb is not None:
                tb = tapb[0] * 3 + tapb[1]
                lw[np.arange(128), m, t, 1, np.arange(128)] = w9qk[128 * m:128 * m + 128, tb]

    # v-path: transposed pw weights + band matrices
    wv = w_pw[2 * C:]                         # [192 vc, 192 c]
    wvt = np.ascontiguousarray(wv.T)          # [c, vc]
    w9v = w9[2 * C:]                          # [192, 9]
    bands = np.zeros((128, 192, 3, 128), np.float32)   # [w_in, vc, dy, w_out]
    for dy in range(3):
        for dx in range(3):
            wp = np.arange(128)
            w_in = wp + dx - 1
            ok = (w_in >= 0) & (w_in < 128)
            bands[w_in[ok][:, None], np.arange(192)[None, :], dy, wp[ok][:, None]] = \
                w9v[:, 3 * dy + dx][None, :]

    wjt = np.ascontiguousarray(w_proj.T)      # [vch, out]
    shared = {
        "w8": w8.astype(f8),
        "lwdw": lw.astype(f8),
        "wvta": wvt[0:128].astype(f16),
        "wvtb": wvt[128:192].astype(f16),
        "bands": bands.astype(f16),
        "wjh": wjt.reshape(4, 48, C).transpose(1, 0, 2).astype(f16),
        "id16": np.eye(128, dtype=f16),
        "id96": np.eye(96, dtype=np.float32),
    }
    maps = []
    for b in range(B):
        m = dict(shared)
        xb = x[b].reshape(C, HH, WW)
        x8 = np.empty((96, 2, HH, WW), np.float32)
        x8[:, 0] = xb[0:96]
        x8[:, 1] = xb[96:192]
        m["x8"] = x8.astype(f8)
        m["x16a"] = xb[0:128].reshape(128, N).astype(f16)
        m["x16b"] = xb[128:192].reshape(64, N).astype(f16)
        maps.append(m)
    return maps


def kernel(x, w_pw, w_dw, w_proj, temperature, num_heads):
    x = np.asarray(x)
    w_pw = np.asarray(w_pw)
    w_dw = np.asarray(w_dw)
    w_proj = np.asarray(w_proj)
    temp = float(np.asarray(temperature))
    assert int(num_heads) == HEADS and x.shape == (B, C, HH, WW)

    key = ("prog", temp)
    if key not in _CACHE:
        _CACHE[key] = build_program(1.0 / temp)
    nc = _CACHE[key]

    in_maps = _host_inputs(x, w_pw, w_dw, w_proj)
    res = run_bass_kernel_spmd(nc, in_maps, core_ids=list(range(8)))
    out = np.stack([res.results[b]["out"].astype(np.float32).reshape(C, HH, WW)
                    for b in range(B)])
    return out


if __name__ == "__main__":
    rng = np.random.default_rng(0)
    x = rng.standard_normal((B, C, HH, WW), dtype=np.float32)
    w_pw = rng.standard_normal((C3, C), dtype=np.float32) * C ** -0.5
    w_dw = rng.standard_normal((C3, 1, 3, 3), dtype=np.float32) / 3.0
    w_proj = rng.standard_normal((C, C), dtype=np.float32) * C ** -0.5
    y = kernel(x, w_pw, w_dw, w_proj, np.float32((C / HEADS) ** -0.5), HEADS)
    print("out", y.shape, y.dtype, float(np.abs(y).max()))

